# revision 15
# baseline (speedup 1.0000x reference)
"""Trainium2 Bass kernel for nn_DiffusionLoss (smoothed-LDDT diffusion loss).

Strategy
--------
The dominant cost is the smoothed-LDDT term: for every unordered pair (i<j)
of the L=4096 tokens-with-coordinates, four sigmoids of |pred_d - gt_d| are
accumulated, per diffusion sample d (D=4).

Host side (inside kernel()):
  * Rows/cols with crd_mask == 0 contribute nothing (mask multiplies both
    numerator and denominator terms), so we compact to the ~L/2 active rows.
  * Since tok_idx is sorted, the combined pair mask
        (j > i) & (tok_i != tok_j)
    over the compacted index space is exactly (j >= hi_i), where hi_i is the
    end of row i's token run -- a per-row column threshold.
  * The upper-triangular pair matrix is cut into [128 x 512] units
    (row-block x column-window). Units are round-robined over the 8 cores,
    padded with dummy units so every core runs an identical program (SPMD).
  * dist^2 is computed on the PE as a K=5 matmul:
        lhsT = [-2x, -2y, -2z, |p_i|^2, 1],  rhs = [x_j, y_j, z_j, 1, |p_j|^2]
    so  psum[i,j] = |p_i|^2 + |p_j|^2 - 2 p_i.p_j = dist^2(i,j).

Device side (per core, Tile-scheduled):
  Phase 1 (gt): sqrt(dist^2) -> gt; build mask m = (gt >= cutoff_i) | (j < hi_i)
    (is_ge / is_lt / max on DVE); G = gt - BIG*m; accumulate sum(m) for the
    denominator. Masked pairs get G ~ -1000 so that later sigma(c-|u|) == 0.
  Phase 2 (pred): sqrt(dist^2) -> pred; delta = |pred - G| (DVE), stored.
  Phase 3: s_c = sigmoid(c - delta) on ACT with accum_out capturing the
    per-partition sum -- the numerator needs no extra vector work.
  Phases are ordered so the ACT table set switches exactly once
  (sqrt_and_others -> sigmoid_and_others).

Host combines per-core partial sums in float64 and adds the (tiny, O(L))
weighted-MSE term computed on host, mirroring the reference formulas.
"""

import math

import numpy as np

import concourse.bacc as bacc
import concourse.bass as bass
import concourse.mybir as mybir
import concourse.tile as tile
from concourse.bass_utils import run_bass_kernel_spmd
from concourse.tile import add_dep_helper

P = 128          # partitions (rows per block)
W = 512          # column window (one fp32 PSUM bank)
D = 4            # diffusion batch
NCORES = 8
BIG = 1000.0     # mask offset pushed into G
SQB = 1e-4       # sqrt bias: sqrt(dist^2 + SQB) guards fp32-rounded negatives
PADC = 100.0     # pad-column marker distance: > cutoff (masked), != BIG (so G stays ~ -900)
SIGC = (0.5, 1.0, 2.0, 4.0)

WEIGHT = 4.0
SIGMA_DATA = 16.0
ALPHA_DNA = 5.0
ALPHA_RNA = 5.0
ALPHA_LIG = 10.0

_prog_cache: dict[int, bass.Bass] = {}


def _build_program(S: int) -> bass.Bass:
    """Bass/Tile program: S units of [P x W] pairs, D diffusion samples."""
    nc = bacc.Bacc(None, target_bir_lowering=False)
    f32 = mybir.dt.float32
    AF = mybir.ActivationFunctionType
    OP = mybir.AluOpType

    # One [5, F5] tensor for all matmul operands (single DMA -> the PE's
    # LoadWeights carries a single semaphore wait; 2+ waits fail codegen),
    # and one [P, F128] tensor for per-partition scalars + iota row.
    F5 = S * P + S * D * P + S * W + S * D * W
    F128 = 2 * S + W
    OFF_LD = S * P
    OFF_RGT = OFF_LD + S * D * P
    OFF_RD = OFF_RGT + S * W
    big5 = nc.dram_tensor("big5", [5, F5], f32, kind="ExternalInput")
    big128 = nc.dram_tensor("big128", [P, F128], f32, kind="ExternalInput")
    out = nc.dram_tensor("out", [P, 8], f32, kind="ExternalOutput")

    with tile.TileContext(nc) as tc:
        with (
            tc.tile_pool(name="singles", bufs=1) as singles,
            tc.tile_pool(name="work", bufs=3) as work,
            tc.tile_pool(name="sig", bufs=4) as sig_pool,
            tc.tile_pool(name="psum", bufs=4, space="PSUM") as psum,
        ):
            big5_sb = singles.tile([5, F5], f32)
            nc.sync.dma_start(out=big5_sb, in_=big5[:, :])
            big128_sb = singles.tile([P, F128], f32)
            nc.sync.dma_start(out=big128_sb, in_=big128[:, :])

            def lgt(s):
                return big5_sb[:, s * P : (s + 1) * P]

            def ld(s, d):
                o = OFF_LD + (s * D + d) * P
                return big5_sb[:, o : o + P]

            def rgt(s):
                o = OFF_RGT + s * W
                return big5_sb[:, o : o + W]

            def rd(s, d):
                o = OFF_RD + (s * D + d) * W
                return big5_sb[:, o : o + W]

            def cut_ap(s):
                return big128_sb[:, 2 * s : 2 * s + 1]

            def hi_ap(s):
                return big128_sb[:, 2 * s + 1 : 2 * s + 2]

            iota_sb = big128_sb[:, 2 * S : 2 * S + W]

            G = singles.tile([P, S, W], f32)
            delta = singles.tile([P, D, S, W], f32)
            nacc = singles.tile([P, D, S, len(SIGC)], f32)
            macc = singles.tile([P, S], f32)

            # per-partition bias constants for the activations
            sqb_t = singles.tile([P, 1], f32)
            nc.vector.memset(sqb_t, SQB)
            c_ts = []
            for c in SIGC:
                ct = singles.tile([P, 1], f32, tag=f"c{c}")
                nc.vector.memset(ct, float(c))
                c_ts.append(ct)

            sqrt_insts = []
            # ---- phase 1: gt distances, masks, G ----
            for s in range(S):
                pg = psum.tile([P, W], f32, tag="ps")
                nc.tensor.matmul(
                    pg, lhsT=lgt(s), rhs=rgt(s), start=True, stop=True,
                )
                gt_t = work.tile([P, W], f32, tag="gt")
                sqrt_insts.append(
                    nc.scalar.activation(gt_t, pg, AF.Sqrt, bias=sqb_t[:, 0:1])
                )
                c2 = work.tile([P, W], f32, tag="c2")
                nc.vector.tensor_scalar(
                    c2, iota_sb, hi_ap(s), None, OP.is_lt
                )
                m = work.tile([P, W], f32, tag="m")
                nc.vector.scalar_tensor_tensor(
                    m, gt_t, cut_ap(s), c2, OP.is_ge, OP.max,
                    accum_out=macc[:, s : s + 1],
                )
                nc.vector.scalar_tensor_tensor(
                    G[:, s, :], m, -BIG, gt_t, OP.mult, OP.add
                )

            # ---- phase 2: pred distances, delta = |pred - G| ----
            for s in range(S):
                for d in range(D):
                    pp = psum.tile([P, W], f32, tag="ps")
                    nc.tensor.matmul(
                        pp, lhsT=ld(s, d), rhs=rd(s, d), start=True, stop=True,
                    )
                    pr = work.tile([P, W], f32, tag="pred")
                    sqrt_insts.append(
                        nc.scalar.activation(pr, pp, AF.Sqrt, bias=sqb_t[:, 0:1])
                    )
                    dsl = delta[:, d, s, :]
                    nc.vector.scalar_tensor_tensor(
                        dsl, G[:, s, :], -1.0, pr, OP.mult, OP.add
                    )
                    # |u| = clear the fp32 sign bit (abs_max is sim-only)
                    dsl_u = dsl.bitcast(mybir.dt.uint32)
                    nc.vector.tensor_scalar(
                        dsl_u, dsl_u, 0x7FFFFFFF, None, OP.bitwise_and
                    )

            # ---- phase 3: sigmoids with accumulated sums ----
            sig_insts = []
            for s in range(S):
                for d in range(D):
                    for ci, c in enumerate(SIGC):
                        st = sig_pool.tile([P, W], f32, tag="sig")
                        sig_insts.append(
                            nc.scalar.activation(
                                st, delta[:, d, s, :], AF.Sigmoid,
                                bias=c_ts[ci][:, 0:1], scale=-1.0,
                                accum_out=nacc[:, d, s, ci : ci + 1],
                            )
                        )
            # Keep every sigmoid after every sqrt on the ACT engine so the
            # activation-table set is switched exactly once.
            for si in sqrt_insts:
                add_dep_helper(sig_insts[0].ins, si.ins, False, "act table phase order")
            for sg in sig_insts[1:]:
                add_dep_helper(sg.ins, sig_insts[0].ins, False, "act table phase order")

            # ---- reductions + output ----
            outt = singles.tile([P, 8], f32)
            nc.vector.memset(outt, 0.0)
            nc.vector.tensor_reduce(
                outt[:, 0:4], nacc, axis=mybir.AxisListType.XY, op=OP.add
            )
            nc.vector.tensor_reduce(
                outt[:, 4:5], macc, axis=mybir.AxisListType.X, op=OP.add
            )
            nc.sync.dma_start(out=out[:, :], in_=outt)
    nc.finalize()
    return nc


def _prep_core_inputs(units, Xgt_a, X_a, cutoff, hi, La):
    """Build the DRAM input arrays for one core.

    units: list of (row_block, col_start) or None (dummy), length S.
    La: number of real (active) rows; columns >= La are masked via BIG.
    Xgt_a: [Lp, 3] compacted+padded gt coords; X_a: [D, Lp, 3].
    cutoff: [Lp] (-1 for pad rows), hi: [Lp] token-run end per row.
    """
    S = len(units)
    La = int(La)
    lhs_gt = np.zeros((S, 5, P), np.float32)
    lhs_d = np.zeros((S, D, 5, P), np.float32)
    rhs_gt = np.zeros((S, 5, W), np.float32)
    rhs_d = np.zeros((S, D, 5, W), np.float32)
    scal = np.zeros((S, 2, P), np.float32)

    rgt_full = Xgt_a.astype(np.float64)
    r_gt = (rgt_full**2).sum(-1)  # [Lp]
    rx_full = X_a.astype(np.float64)
    r_x = (rx_full**2).sum(-1)  # [D, Lp]

    for s, u in enumerate(units):
        if u is None:
            scal[s, 0, :] = -1.0
            rhs_gt[s, 4, :] = PADC * PADC
            continue
        b, c0 = u
        rows = slice(b * P, b * P + P)
        # lhsT = [-2x, -2y, -2z, r_i, 1]
        lhs_gt[s, 0:3, :] = -2.0 * rgt_full[rows].T
        lhs_gt[s, 3, :] = r_gt[rows]
        lhs_gt[s, 4, :] = 1.0
        lhs_d[s, :, 0:3, :] = -2.0 * rx_full[:, rows].transpose(0, 2, 1)
        lhs_d[s, :, 3, :] = r_x[:, rows]
        lhs_d[s, :, 4, :] = 1.0

        ncols = max(0, min(W, La - c0))
        cols = slice(c0, c0 + ncols)
        # rhs = [x, y, z, 1, r_j]; pad cols of rhs_gt get r = PADC^2 -> masked
        rhs_gt[s, 0:3, :ncols] = rgt_full[cols].T
        rhs_gt[s, 3, :ncols] = 1.0
        rhs_gt[s, 4, :ncols] = r_gt[cols]
        rhs_gt[s, 4, ncols:] = PADC * PADC
        rhs_d[s, :, 0:3, :ncols] = rx_full[:, cols].transpose(0, 2, 1)
        rhs_d[s, :, 3, :ncols] = 1.0
        rhs_d[s, :, 4, :ncols] = r_x[:, cols]

        scal[s, 0, :] = cutoff[rows]
        scal[s, 1, :] = hi[rows] - c0

    # Pack into the two device tensors (see _build_program offsets).
    big5 = np.concatenate(
        [
            lhs_gt.transpose(1, 0, 2).reshape(5, S * P),
            lhs_d.transpose(2, 0, 1, 3).reshape(5, S * D * P),
            rhs_gt.transpose(1, 0, 2).reshape(5, S * W),
            rhs_d.transpose(2, 0, 1, 3).reshape(5, S * D * W),
        ],
        axis=1,
    ).astype(np.float32)
    big128 = np.concatenate(
        [
            scal.transpose(2, 0, 1).reshape(P, 2 * S),
            np.broadcast_to(np.arange(W, dtype=np.float32), (P, W)),
        ],
        axis=1,
    ).astype(np.float32)
    return {"big5": np.ascontiguousarray(big5),
            "big128": np.ascontiguousarray(big128)}


def _plan(La: int):
    """Unit list + per-core assignment for La active rows."""
    Lp = ((La + P - 1) // P) * P
    n_blocks = Lp // P
    units = []
    for b in range(n_blocks):
        span = Lp - b * P
        for k in range(math.ceil(span / W)):
            units.append((b, b * P + k * W))
    S = math.ceil(len(units) / NCORES)
    padded = units + [None] * (S * NCORES - len(units))
    per_core = [padded[c::NCORES] for c in range(NCORES)]
    return Lp, S, per_core


def kernel(**inputs: np.ndarray) -> np.ndarray:
    X_L = np.asarray(inputs["X_L"]).astype(np.float32)          # [D, L, 3]
    X_gt_L = np.asarray(inputs["X_gt_L"]).astype(np.float32)    # [1, L, 3]
    crd = np.asarray(inputs["crd_mask_L"]).astype(bool)[0]      # [L]
    is_dna = np.asarray(inputs["is_dna"]).astype(bool)
    is_rna = np.asarray(inputs["is_rna"]).astype(bool)
    is_lig = np.asarray(inputs["is_ligand"]).astype(bool)
    tok = np.asarray(inputs["tok_idx"]).astype(np.int64)        # [L]
    t = np.asarray(inputs["t"]).astype(np.float64)              # [D]

    X_gt = np.nan_to_num(X_gt_L)[0]  # [L, 3]

    # ---------- lddt term: compact to crd-active rows ----------
    act = np.flatnonzero(crd)
    La = len(act)
    Lp, S, per_core = _plan(La)

    Xgt_a = np.zeros((Lp, 3), np.float32)
    Xgt_a[:La] = X_gt[act]
    X_a = np.zeros((D, Lp, 3), np.float32)
    X_a[:, :La] = X_L[:, act]
    tok_a = tok[act]
    hi = np.zeros(Lp, np.float32)
    hi[:La] = np.searchsorted(tok_a, tok_a, side="right").astype(np.float32)
    is_na = (is_dna | is_rna)[tok_a]
    cutoff = np.full(Lp, -1.0, np.float32)
    cutoff[:La] = np.where(is_na, 30.0, 15.0)

    nc = _prog_cache.get(S)
    if nc is None:
        nc = _build_program(S)
        _prog_cache[S] = nc

    in_maps = [
        _prep_core_inputs(per_core[c], Xgt_a, X_a, cutoff, hi, La)
        for c in range(NCORES)
    ]
    res = run_bass_kernel_spmd(nc, in_maps, core_ids=list(range(NCORES)))

    numer = np.zeros(D, np.float64)
    m_tot = 0.0
    for r in res.results:
        o = r["out"].astype(np.float64)
        numer += o[:, 0:4].sum(0)
        m_tot += o[:, 4].sum()
    denom = NCORES * S * (P * W) - m_tot
    lddt = 0.25 * numer / (denom + 1e-6)
    lddt_loss = (1.0 - lddt).mean()

    # ---------- mse term (O(L), host) ----------
    mask = crd.astype(np.float64)
    alpha = (
        is_dna * ALPHA_DNA + is_rna * ALPHA_RNA + is_lig * ALPHA_LIG
    ).astype(np.float64)
    w_L = (1.0 + alpha[tok]) * mask  # [L]
    sq = ((X_L.astype(np.float64) - X_gt.astype(np.float64)[None]) ** 2).sum(-1)
    l_mse = (1.0 / 3.0) * (w_L[None] * sq).sum(-1) / (mask.sum() + 1e-4)
    lam = (t**2 + SIGMA_DATA**2) / ((t * SIGMA_DATA) ** 2)
    l_diff = np.minimum(lam * l_mse, 2.0)

    total = WEIGHT * (l_diff.mean() + lddt_loss)
    return np.asarray(total, dtype=np.float32)
